# revision 9
# baseline (speedup 1.0000x reference)
"""GPT-2-small (B=2,T=1024,E=768,L=12,H=12,V=50304) forward on 8 trn2 NeuronCores.

Sharding: DP=2 over batch (cores 0-3 = batch0, 4-7 = batch1); within a group,
sequence-parallel over tokens: core (g, r) owns canonical 128-token chunks
(r, 7-r) of its batch. All row-wise ops (LN, QKV, FFN, proj) are token-local
with full weights streamed from HBM; attention gathers K/V within the group
via two AllGathers per layer (hidden behind compute).  lm_head is
vocab-parallel: each core computes its batch x 12576 vocab columns.

The SPMD program is identical on all 8 cores; per-core differences enter only
through input data (token ids, pos rows, causal mask tables, lm_w slice).
Matmuls run in bf16 with fp32 accumulation; the residual stream, layernorm
statistics and softmax accumulation stay fp32.
"""

import numpy as np
import ml_dtypes

import concourse.bacc as bacc
import concourse.bass as bass
import concourse.tile as tile
import concourse.mybir as mybir
from concourse.bass import ds, ts
from concourse.bass_utils import run_bass_kernel_spmd

F32 = mybir.dt.float32
BF16 = mybir.dt.bfloat16
AF = mybir.ActivationFunctionType
OP = mybir.AluOpType

B, T, V, E, L, H = 2, 1024, 50304, 768, 12, 12
HS = 64
P = 128
KO = 6            # E / 128
FCK = 24          # 3072 / 128
VS = V // 4       # 12576 vocab shard
VPAD = 12800      # padded to 25*512
NLM = 25          # lm chunks of 512
RG = [[0, 1, 2, 3], [4, 5, 6, 7]]
EPS = 1e-5

_cache = {}


def _build():
    nc = bacc.Bacc("TRN2", target_bir_lowering=False, debug=False, num_devices=8)

    # ---------------- DRAM I/O ----------------
    idx_d = nc.dram_tensor("idx", [256], mybir.dt.int32, kind="ExternalInput").ap()
    temb_d = nc.dram_tensor("temb", [V, E], BF16, kind="ExternalInput").ap()
    pos_d = nc.dram_tensor("pos", [256, E], BF16, kind="ExternalInput").ap()
    mask_d = nc.dram_tensor("masks", [12, P, P], BF16, kind="ExternalInput").ap()
    wq_d = nc.dram_tensor("wq", [L, P, KO, E], BF16, kind="ExternalInput").ap()
    wk_d = nc.dram_tensor("wk", [L, P, KO, E], BF16, kind="ExternalInput").ap()
    wv_d = nc.dram_tensor("wv", [L, P, KO, E], BF16, kind="ExternalInput").ap()
    wp_d = nc.dram_tensor("wp", [L, P, KO, E], BF16, kind="ExternalInput").ap()
    wfc_d = nc.dram_tensor("wfc", [L, 4, P, KO, E], BF16, kind="ExternalInput").ap()
    w2_d = nc.dram_tensor("w2", [L, FCK, P, E], BF16, kind="ExternalInput").ap()
    bqk_d = nc.dram_tensor("bqk", [L, 12, P], F32, kind="ExternalInput").ap()
    bfc_d = nc.dram_tensor("bfc", [L, FCK, P], F32, kind="ExternalInput").ap()
    brow_d = nc.dram_tensor("brow", [L, 1, 3 * E], F32, kind="ExternalInput").ap()
    lmw_d = nc.dram_tensor("lmw", [P, KO, VPAD], BF16, kind="ExternalInput").ap()
    lmb_d = nc.dram_tensor("lmb", [1, VPAD], F32, kind="ExternalInput").ap()
    out_d = nc.dram_tensor("logits", [1024, VS], F32, kind="ExternalOutput").ap()

    with tile.TileContext(nc) as tc:
        from contextlib import ExitStack
        gctx = ExitStack()
        # ---------------- pools ----------------
        singles = gctx.enter_context(tc.tile_pool(name="singles", bufs=1))
        pstat = gctx.enter_context(tc.tile_pool(name="pstat", bufs=4))
        pact = gctx.enter_context(tc.tile_pool(name="pact", bufs=2))
        pbias = gctx.enter_context(tc.tile_pool(name="pbias", bufs=1))
        dram = gctx.enter_context(tc.tile_pool(name="dram", bufs=1, space="DRAM"))

        lctx = ExitStack()
        pw = lctx.enter_context(tc.tile_pool(name="pw", bufs=5))
        pw2 = lctx.enter_context(tc.tile_pool(name="pw2", bufs=6))
        pqkv = lctx.enter_context(tc.tile_pool(name="pqkv", bufs=2))
        pkv = lctx.enter_context(tc.tile_pool(name="pkv", bufs=1))
        patt = lctx.enter_context(tc.tile_pool(name="patt", bufs=6))
        phT = lctx.enter_context(tc.tile_pool(name="phT", bufs=1))
        ps_s = lctx.enter_context(tc.tile_pool(name="ps_s", bufs=3, space="PSUM"))
        ps_o = lctx.enter_context(tc.tile_pool(name="ps_o", bufs=2, space="PSUM"))
        ps_big = lctx.enter_context(tc.tile_pool(name="ps_big", bufs=3, space="PSUM"))

        # ---------------- constants / setup ----------------
        eps_sb = singles.tile([P, 1], F32, name="eps_sb")
        nc.vector.memset(eps_sb[:], EPS)
        ones_sb = singles.tile([1, P], F32, name="ones_sb")
        nc.vector.memset(ones_sb[:], 1.0)
        mask_sb = singles.tile([P, 12, P], BF16, name="mask_sb")
        nc.sync.dma_start(mask_sb[:], mask_d.rearrange("s k q -> k s q"))
        idx_sb = singles.tile([P, 2], mybir.dt.int32, name="idx_sb")
        nc.sync.dma_start(idx_sb[:], idx_d.rearrange("(c p) -> p c", p=P))
        pos_sb = singles.tile([P, 2, E], BF16, name="pos_sb")
        nc.sync.dma_start(pos_sb[:], pos_d.rearrange("(c p) m -> p c m", p=P))

        # residual stream x: [128 tok, 2 chunks, 768] fp32, persistent
        x = singles.tile([P, 2, E], F32, name="x_res")
        emb_sb = singles.tile([P, 2, E], BF16, name="emb_sb")
        for c in range(2):
            nc.gpsimd.indirect_dma_start(
                out=emb_sb[:, c, :], out_offset=None,
                in_=temb_d,
                in_offset=bass.IndirectOffsetOnAxis(ap=idx_sb[:, c:c + 1], axis=0),
            )
            nc.vector.tensor_tensor(x[:, c, :], emb_sb[:, c, :], pos_sb[:, c, :], OP.add)

        def layernorm(xin, xout):
            """xin fp32 [128,2,768] -> xout bf16 [128,2,768] (pure (x-m)*rstd)."""
            for c in range(2):
                st = pstat.tile([P, 2, 6], F32, tag="st")
                xv = xin[:, c, :].rearrange("p (a b) -> p a b", b=384)
                for sg in range(2):
                    nc.vector.bn_stats(st[:, sg, :], xv[:, sg, :])
                mv = pstat.tile([P, 2], F32, tag="mv")
                nc.vector.bn_aggr(mv[:], st[:])
                rstd = pstat.tile([P, 1], F32, tag="rs")
                nc.scalar.activation(rstd[:], mv[:, 1:2], AF.Sqrt, bias=eps_sb[:], scale=1.0)
                nc.vector.reciprocal(rstd[:], rstd[:])
                nmr = pstat.tile([P, 1], F32, tag="nm")
                nc.vector.tensor_tensor(nmr[:], mv[:, 0:1], rstd[:], OP.mult)
                nc.vector.tensor_scalar_mul(nmr[:], nmr[:], -1.0)
                nc.scalar.activation(xout[:, c, :], xin[:, c, :], AF.Identity,
                                     bias=nmr[:], scale=rstd[:])

        def transpose_act(xh, tag):
            """bf16 [128,2,768] token-major -> [128,6,256] feature-major."""
            xhT = pact.tile([P, KO, 256], BF16, tag=tag)
            for c in range(2):
                for f in range(KO):
                    nc.sync.dma_start_transpose(
                        xhT[:, f, ts(c, P)], xh[:, c, ts(f, P)])
            return xhT

        # ---------------- transformer layers ----------------
        for l in range(L):
            wq_sb = pw.tile([P, KO, E], BF16, tag="w")
            nc.sync.dma_start(wq_sb[:], wq_d[l])
            wk_sb = pw.tile([P, KO, E], BF16, tag="w")
            nc.sync.dma_start(wk_sb[:], wk_d[l])
            wv_sb = pw.tile([P, KO, E], BF16, tag="w")
            nc.sync.dma_start(wv_sb[:], wv_d[l])
            wp_sb = pw.tile([P, KO, E], BF16, tag="w")
            nc.sync.dma_start(wp_sb[:], wp_d[l])
            bqk_sb = pbias.tile([P, 12], F32, tag="bqk")
            nc.sync.dma_start(bqk_sb[:], bqk_d[l].rearrange("a p -> p a"))
            bfc_sb = pbias.tile([P, FCK], F32, tag="bfc")
            nc.sync.dma_start(bfc_sb[:], bfc_d[l].rearrange("a p -> p a"))
            brow_sb = pbias.tile([1, 3 * E], F32, tag="brow")
            nc.sync.dma_start(brow_sb[:], brow_d[l])

            # LN1 + transpose
            xh = pact.tile([P, 2, E], BF16, tag="xh")
            layernorm(x, xh)
            xhT = transpose_act(xh, "xhT")

            # Q,K projections (feature-major outputs)
            qT = pqkv.tile([P, KO, 256], BF16, tag="qT")
            kT_loc = pqkv.tile([P, KO, 256], BF16, tag="kT")
            for m in range(KO):
                for dst, w_sb, boff in ((qT, wq_sb, 0), (kT_loc, wk_sb, 6)):
                    pm = ps_big.tile([P, 512], F32, tag="big")
                    for kk in range(KO):
                        nc.tensor.matmul(pm[:, :256], w_sb[:, kk, ts(m, P)],
                                         xhT[:, kk, :], start=(kk == 0), stop=(kk == 5))
                    nc.scalar.activation(dst[:, m, :], pm[:, :256], AF.Identity,
                                         bias=bqk_sb[:, boff + m:boff + m + 1], scale=1.0)

            # AllGather K
            agk_i = dram.tile([P, KO * 256], BF16, name=f"agki{l}")
            agk_o = dram.tile([4, P, KO * 256], BF16, name=f"agko{l}")
            nc.sync.dma_start(agk_i[:].rearrange("p (a b) -> p a b", a=KO), kT_loc[:])
            nc.gpsimd.collective_compute(
                "AllGather", OP.bypass, replica_groups=RG,
                ins=[agk_i[:].opt()], outs=[agk_o[:].opt()])
            kT_all = pkv.tile([P, KO, 8, P], BF16, tag="kTa")
            for rr in range(4):
                kv = agk_o[rr].rearrange("p (a b) -> p a b", a=KO)
                for cc in range(2):
                    canon = rr if cc == 0 else 7 - rr
                    nc.sync.dma_start(kT_all[:, :, canon, :], kv[:, :, ts(cc, P)])

            # V projection (token-major) + ones column
            vplus = pqkv.tile([P, 2, H, HS + 1], BF16, tag="vpl")
            nc.vector.memset(vplus[:, :, :, HS:HS + 1], 1.0)
            for tt in range(2):
                for c0, cw in ((0, 512), (512, 256)):
                    pm = ps_big.tile([P, 512], F32, tag="big")
                    nc.tensor.matmul(pm[:, :cw], ones_sb[:, :P],
                                     brow_sb[:, c0:c0 + cw], start=True, stop=False)
                    for kk in range(KO):
                        nc.tensor.matmul(pm[:, :cw], xhT[:, kk, ts(tt, P)],
                                         wv_sb[:, kk, c0:c0 + cw],
                                         start=False, stop=(kk == 5))
                    nc.vector.tensor_copy(
                        vplus[:, tt, c0 // HS:(c0 + cw) // HS, 0:HS],
                        pm[:, :cw].rearrange("p (h d) -> p h d", d=HS))

            # AllGather V
            agv_i = dram.tile([P, 2 * H * (HS + 1)], BF16, name=f"agvi{l}")
            agv_o = dram.tile([4, P, 2 * H * (HS + 1)], BF16, name=f"agvo{l}")
            nc.sync.dma_start(
                agv_i[:].rearrange("p (c h d) -> p c h d", c=2, h=H), vplus[:])
            nc.gpsimd.collective_compute(
                "AllGather", OP.bypass, replica_groups=RG,
                ins=[agv_i[:].opt()], outs=[agv_o[:].opt()])
            vplus_all = pkv.tile([P, 8, H, HS + 1], BF16, tag="vpa")
            for rr in range(4):
                vv = agv_o[rr].rearrange("p (c h d) -> p c h d", c=2, h=H)
                for cc in range(2):
                    canon = rr if cc == 0 else 7 - rr
                    nc.sync.dma_start(vplus_all[:, canon, :, :], vv[:, cc])

            # ---------------- attention ----------------
            attT = patt.tile([P, KO, 256], BF16, tag="attT", bufs=2)
            for h in range(H):
                base = HS * (h % 2)
                sub = h // 2
                psA = ps_s.tile([P, 512], F32, tag="s")
                psB1 = ps_s.tile([P, 512], F32, tag="s")
                psB2 = ps_s.tile([P, 512], F32, tag="s")
                for j in range(4):
                    nc.tensor.matmul(psA[:, ts(j, P)],
                                     kT_all[base:base + HS, sub, j, :],
                                     qT[base:base + HS, sub, 0:P],
                                     start=True, stop=True)
                for j in range(8):
                    tgt = psB1 if j < 4 else psB2
                    nc.tensor.matmul(tgt[:, ts(j % 4, P)],
                                     kT_all[base:base + HS, sub, j, :],
                                     qT[base:base + HS, sub, P:256],
                                     start=True, stop=True)
                pA = patt.tile([P, 4, P], BF16, tag="pt")
                pB1 = patt.tile([P, 4, P], BF16, tag="pt")
                pB2 = patt.tile([P, 4, P], BF16, tag="pt")
                for src, dst in ((psA, pA), (psB1, pB1), (psB2, pB2)):
                    nc.scalar.activation(dst[:], src[:].rearrange("p (a b) -> p a b", a=4),
                                         AF.Exp, scale=HS ** -0.5)
                nc.vector.tensor_tensor(pA[:], pA[:], mask_sb[:, 0:4, :], OP.mult)
                nc.vector.tensor_tensor(pB1[:], pB1[:], mask_sb[:, 4:8, :], OP.mult)
                nc.vector.tensor_tensor(pB2[:], pB2[:], mask_sb[:, 8:12, :], OP.mult)
                pso = ps_o.tile([HS + 1, 256], F32, tag="o")
                for j in range(4):
                    nc.tensor.matmul(pso[:, 0:P], vplus_all[:, j, h, :],
                                     pA[:, j, :], start=(j == 0), stop=(j == 3))
                for j in range(8):
                    pt = pB1 if j < 4 else pB2
                    nc.tensor.matmul(pso[:, P:256], vplus_all[:, j, h, :],
                                     pt[:, j % 4, :], start=(j == 0), stop=(j == 7))
                rc = patt.tile([1, 256], F32, tag="rc", bufs=4)
                nc.vector.reciprocal(rc[:], pso[HS:HS + 1, :])
                rcd = dram.tile([1, 256], F32, tag="rcd", bufs=2)
                nc.sync.dma_start(rcd[:], rc[:])
                rcb = patt.tile([HS, 256], F32, tag="rcb", bufs=2)
                rcd_ap = rcd[:]
                bcast = bass.AP(tensor=rcd_ap.tensor, offset=rcd_ap.offset,
                                ap=[[0, HS]] + [list(p) for p in rcd_ap.ap][1:])
                nc.sync.dma_start(rcb[:], bcast)
                if base == 0:
                    nc.vector.tensor_tensor(attT[0:HS, sub, :], pso[0:HS, :],
                                            rcb[:], OP.mult)
                else:
                    ot = patt.tile([HS, 256], BF16, tag="ot", bufs=2)
                    nc.vector.tensor_tensor(ot[:], pso[0:HS, :], rcb[:], OP.mult)
                    nc.sync.dma_start(attT[HS:P, sub, :], ot[:])

            # output projection + residual
            for tt in range(2):
                for c0, cw in ((0, 512), (512, 256)):
                    pm = ps_big.tile([P, 512], F32, tag="big")
                    nc.tensor.matmul(pm[:, :cw], ones_sb[:, :P],
                                     brow_sb[:, E + c0:E + c0 + cw], start=True, stop=False)
                    for kk in range(KO):
                        nc.tensor.matmul(pm[:, :cw], attT[:, kk, ts(tt, P)],
                                         wp_sb[:, kk, c0:c0 + cw],
                                         start=False, stop=(kk == 5))
                    nc.vector.tensor_tensor(x[:, tt, c0:c0 + cw], x[:, tt, c0:c0 + cw],
                                            pm[:, :cw], OP.add)

            # ---------------- FFN ----------------
            xh2 = pact.tile([P, 2, E], BF16, tag="xh")
            layernorm(x, xh2)
            xh2T = transpose_act(xh2, "xhT")
            hT = phT.tile([P, FCK, 256], BF16, tag="hT")
            for ci in range(4):
                wfc_sb = pw.tile([P, KO, E], BF16, tag="w")
                nc.sync.dma_start(wfc_sb[:], wfc_d[l, ci])
                for mm in range(KO):
                    ch = ci * KO + mm
                    pm = ps_big.tile([P, 512], F32, tag="big")
                    for kk in range(KO):
                        nc.tensor.matmul(pm[:, :256], wfc_sb[:, kk, ts(mm, P)],
                                         xh2T[:, kk, :], start=(kk == 0), stop=(kk == 5))
                    nc.scalar.activation(hT[:, ch, :], pm[:, :256], AF.Relu,
                                         bias=bfc_sb[:, ch:ch + 1], scale=1.0)
            for c0, cw in ((0, 512), (512, 256)):
                pms = []
                for tt in range(2):
                    pm = ps_big.tile([P, 512], F32, tag="big")
                    nc.tensor.matmul(pm[:, :cw], ones_sb[:, :P],
                                     brow_sb[:, 2 * E + c0:2 * E + c0 + cw],
                                     start=True, stop=False)
                    pms.append(pm)
                for kk in range(FCK):
                    w2_sb = pw2.tile([P, 512], BF16, tag="w2")
                    nc.sync.dma_start(w2_sb[:, :cw], w2_d[l, kk][:, c0:c0 + cw])
                    for tt in range(2):
                        nc.tensor.matmul(pms[tt][:, :cw], hT[:, kk, ts(tt, P)],
                                         w2_sb[:, :cw], start=False, stop=(kk == FCK - 1))
                for tt in range(2):
                    nc.vector.tensor_tensor(x[:, tt, c0:c0 + cw], x[:, tt, c0:c0 + cw],
                                            pms[tt][:, :cw], OP.add)

        # ---------------- final LN + AllGather + lm_head ----------------
        xhf = pact.tile([P, 2, E], BF16, tag="xh")
        layernorm(x, xhf)
        xhfT = transpose_act(xhf, "xhT")
        agf_i = dram.tile([P, KO * 256], BF16, name="agfi")
        agf_o = dram.tile([4, P, KO * 256], BF16, name="agfo")
        nc.sync.dma_start(agf_i[:].rearrange("p (a b) -> p a b", a=KO), xhfT[:])
        nc.gpsimd.collective_compute(
            "AllGather", OP.bypass, replica_groups=RG,
            ins=[agf_i[:].opt()], outs=[agf_o[:].opt()])
        xfT = pkv.tile([P, KO, 8, P], BF16, tag="kTa")
        for rr in range(4):
            kv = agf_o[rr].rearrange("p (a b) -> p a b", a=KO)
            for cc in range(2):
                canon = rr if cc == 0 else 7 - rr
                nc.sync.dma_start(xfT[:, :, canon, :], kv[:, :, ts(cc, P)])

        lctx.close()

        with tc.tile_pool(name="plm", bufs=3) as plm, \
             tc.tile_pool(name="plog", bufs=6) as plog, \
             tc.tile_pool(name="ps_lm", bufs=6, space="PSUM") as ps_lm:
            for chk in range(NLM):
                lw = plm.tile([P, KO, 512], BF16, tag="lw")
                N = 512 if chk < NLM - 1 else VS - 512 * (NLM - 1)
                nc.sync.dma_start(lw[:], lmw_d[:, :, ts(chk, 512)])
                lmb_sb = plm.tile([1, 512], F32, tag="lmb")
                nc.sync.dma_start(lmb_sb[:, :N], lmb_d[:, 512 * chk:512 * chk + N])
                for tt in range(8):
                    pm = ps_lm.tile([P, 512], F32, tag="lm")
                    nc.tensor.matmul(pm[:, :N], ones_sb[:, :P],
                                     lmb_sb[:, :N], start=True, stop=False)
                    for kk in range(KO):
                        nc.tensor.matmul(pm[:, :N], xfT[:, kk, tt, :],
                                         lw[:, kk, :N], start=False, stop=(kk == 5))
                    lg = plog.tile([P, 512], F32, tag="lg")
                    if tt % 2 == 0:
                        nc.scalar.copy(lg[:, :N], pm[:, :N])
                    else:
                        nc.vector.tensor_copy(lg[:, :N], pm[:, :N])
                    nc.sync.dma_start(out_d[ts(tt, P), 512 * chk:512 * chk + N],
                                      lg[:, :N])
        gctx.close()

    nc.compile()
    return nc


def _prep(inputs):
    bf = ml_dtypes.bfloat16
    f = np.float32
    g = lambda k: np.asarray(inputs[k], f)
    idx = np.asarray(inputs["idx"]).astype(np.int32)
    wq, wk, wv, wproj = g("wq"), g("wk"), g("wv"), g("wproj")
    g1, b1, g2, b2 = g("ln1_g"), g("ln1_b"), g("ln2_g"), g("ln2_b")
    wfc, wpr2 = g("wfc"), g("wpr2")
    bfc, bproj, bpr2 = g("bfc"), g("bproj"), g("bpr2")
    gf, bff = g("lnf_g"), g("lnf_b")
    lm_w, lm_b = g("lm_w"), g("lm_b")

    wq_e = g1[:, :, None] * wq
    wk_e = g1[:, :, None] * wk
    wv_e = g1[:, :, None] * wv
    wfc_e = g2[:, :, None] * wfc
    bq_e = np.einsum("le,leo->lo", b1, wq)
    bk_e = np.einsum("le,leo->lo", b1, wk)
    bv_e = np.einsum("le,leo->lo", b1, wv)
    bfc_e = bfc + np.einsum("le,leo->lo", b2, wfc)
    lmw_e = gf[:, None] * lm_w
    lmb_e = lm_b + bff @ lm_w

    def pack(w):  # [L,768,N] -> [L,128,6,N]
        Lx, Ex, Nx = w.shape
        return np.ascontiguousarray(
            w.reshape(Lx, KO, P, Nx).transpose(0, 2, 1, 3)).astype(bf)

    com = {
        "temb": np.asarray(inputs["tok_emb"], f).astype(bf),
        "wq": pack(wq_e), "wk": pack(wk_e), "wv": pack(wv_e), "wp": pack(wproj),
        "wfc": np.ascontiguousarray(
            pack(wfc_e).reshape(L, P, KO, 4, E).transpose(0, 3, 1, 2, 4)),
        "w2": wpr2.reshape(L, FCK, P, E).astype(bf),
        "bqk": np.concatenate(
            [bq_e.reshape(L, KO, P), bk_e.reshape(L, KO, P)], axis=1).astype(f),
        "bfc": bfc_e.reshape(L, FCK, P).astype(f),
        "brow": np.concatenate([bv_e, bproj, bpr2], axis=1)[:, None, :].astype(f),
    }
    pos = np.asarray(inputs["pos_emb"], f).astype(bf)

    in_maps = []
    ar = np.arange(P)
    for core in range(8):
        gb, r = divmod(core, 4)
        c1, c2 = r, 7 - r
        m = dict(com)
        sl = lmw_e[:, r * VS:(r + 1) * VS]
        lmw_pad = np.zeros((E, VPAD), f)
        lmw_pad[:, :VS] = sl
        m["lmw"] = np.ascontiguousarray(
            lmw_pad.reshape(KO, P, VPAD).transpose(1, 0, 2)).astype(bf)
        lmb_pad = np.zeros((1, VPAD), f)
        lmb_pad[0, :VS] = lmb_e[r * VS:(r + 1) * VS]
        m["lmb"] = lmb_pad
        m["idx"] = np.concatenate(
            [idx[gb, c1 * P:(c1 + 1) * P], idx[gb, c2 * P:(c2 + 1) * P]])
        m["pos"] = np.concatenate(
            [pos[c1 * P:(c1 + 1) * P], pos[c2 * P:(c2 + 1) * P]])
        masks = np.zeros((12, P, P), f)
        for j in range(4):
            masks[j] = (j * P + ar[:, None]) <= (c1 * P + ar[None, :])
        for j in range(8):
            masks[4 + j] = (j * P + ar[:, None]) <= (c2 * P + ar[None, :])
        m["masks"] = masks.astype(bf)
        in_maps.append(m)
    return in_maps


LAST_RESULTS = None
LAST_TIME_NS = None


def _timed_run(nc, in_maps, reps=3):
    """Replicates bass2jax.run_bass_via_pjrt's multi-core path, but keeps
    inputs device-resident so repeated executions time the NEFF itself."""
    import time as _time
    import jax
    from jax.experimental.shard_map import shard_map
    from jax.sharding import Mesh, PartitionSpec, NamedSharding
    from concourse import bass2jax as b2j
    import concourse.mybir as _mb

    b2j.install_neuronx_cc_hook()
    n_cores = len(in_maps)
    partition_name = nc.partition_id_tensor.name if nc.partition_id_tensor else None
    in_names, out_names, out_avals, zero_outs = [], [], [], []
    for alloc in nc.m.functions[0].allocations:
        if not isinstance(alloc, _mb.MemoryLocationSet):
            continue
        name = alloc.memorylocations[0].name
        if alloc.kind == "ExternalInput":
            if name != partition_name:
                in_names.append(name)
        elif alloc.kind == "ExternalOutput":
            out_names.append(name)
            shape = tuple(alloc.tensor_shape)
            dtype = _mb.dt.np(alloc.dtype)
            out_avals.append(jax.core.ShapedArray(shape, dtype))
            zero_outs.append(np.zeros(shape, dtype))
    n_params = len(in_names)
    n_outs = len(out_avals)
    in_names.extend(out_names)
    if partition_name is not None:
        in_names.append(partition_name)
    donate = tuple(range(n_params, n_params + n_outs))

    def _body(*args):
        operands = list(args)
        if partition_name is not None:
            operands.append(b2j.partition_id_tensor())
        return tuple(b2j._bass_exec_p.bind(
            *operands, out_avals=tuple(out_avals), in_names=tuple(in_names),
            out_names=tuple(out_names), lowering_input_output_aliases=(),
            sim_require_finite=True, sim_require_nnan=True, nc=nc))

    devices = jax.devices()[:n_cores]
    mesh = Mesh(np.asarray(devices), ("core",))
    spec = PartitionSpec("core")
    sharded = jax.jit(
        shard_map(_body, mesh=mesh, in_specs=(spec,) * (n_params + n_outs),
                  out_specs=(spec,) * n_outs, check_rep=False),
        donate_argnums=donate, keep_unused=True)
    sh = NamedSharding(mesh, spec)
    concat_in = [
        jax.device_put(
            np.concatenate([np.asarray(in_maps[c][nm]) for c in range(n_cores)], axis=0),
            sh)
        for nm in in_names[:n_params]]
    jax.block_until_ready(concat_in)
    times = []
    out_arrs = None
    for rep in range(reps):
        zeros_dev = [
            jax.device_put(np.zeros((n_cores * z.shape[0], *z.shape[1:]), z.dtype), sh)
            for z in zero_outs]
        jax.block_until_ready(zeros_dev)
        t0 = _time.perf_counter()
        out_arrs = sharded(*concat_in, *zeros_dev)
        jax.block_until_ready(out_arrs)
        times.append(_time.perf_counter() - t0)
    results = [
        {nm: np.asarray(out_arrs[i]).reshape(n_cores, *out_avals[i].shape)[c]
         for i, nm in enumerate(out_names)}
        for c in range(n_cores)]
    return results, times


def kernel(**inputs):
    global LAST_RESULTS, LAST_TIME_NS
    import os
    if "nc" not in _cache:
        _cache["nc"] = _build()
    nc = _cache["nc"]
    in_maps = _prep(inputs)
    reps = int(os.environ.get("KBENCH_TIME_REPS", "0"))
    if reps > 0:
        results, times = _timed_run(nc, in_maps, reps=reps)
        LAST_TIME_NS = int(min(times) * 1e9)
        LAST_RESULTS = None
    else:
        res = run_bass_kernel_spmd(nc, in_maps, core_ids=list(range(8)))
        LAST_RESULTS = res
        results = res.results
    out = np.zeros((B, T, V), np.float32)
    for core in range(8):
        gb, r = divmod(core, 4)
        out[gb, :, r * VS:(r + 1) * VS] = results[core]["logits"]
    return out


# revision 18
# speedup vs baseline: 53.7916x; 53.7916x over previous
"""GPT-2-small (B=2,T=1024,E=768,L=12,H=12,V=50304) forward on 8 trn2 NeuronCores.

Sharding: DP=2 over batch (cores 0-3 = batch0, 4-7 = batch1); within a group,
sequence-parallel over tokens: core (g, r) owns canonical 128-token chunks
(r, 7-r) of its batch. All row-wise ops (LN, QKV, FFN, proj) are token-local
with full weights streamed from HBM; attention gathers K/V within the group
via two AllGathers per layer (hidden behind compute).  lm_head is
vocab-parallel: each core computes its batch x 12576 vocab columns.

The SPMD program is identical on all 8 cores; per-core differences enter only
through input data (token ids, pos rows, causal mask tables, lm_w slice).
Matmuls run in bf16 with fp32 accumulation; the residual stream, layernorm
statistics and softmax accumulation stay fp32.
"""

import numpy as np
import ml_dtypes

import concourse.bacc as bacc
import concourse.bass as bass
import concourse.tile as tile
import concourse.mybir as mybir
from concourse.bass import ds, ts
from concourse.bass_utils import run_bass_kernel_spmd

F32 = mybir.dt.float32
BF16 = mybir.dt.bfloat16
AF = mybir.ActivationFunctionType
OP = mybir.AluOpType

B, T, V, E, L, H = 2, 1024, 50304, 768, 12, 12
HS = 64
P = 128
KO = 6            # E / 128
FCK = 24          # 3072 / 128
VS = V // 4       # 12576 vocab shard
VPAD = 12800      # padded to 25*512
NLM = 25          # lm chunks of 512
RG = [[0, 1, 2, 3], [4, 5, 6, 7]]
EPS = 1e-5

_cache = {}


def _build(inner=1):
    import os as _os
    _NOAG = bool(int(_os.environ.get("KBENCH_NOAG", "0")))
    nc = bacc.Bacc("TRN2", target_bir_lowering=False, debug=False, num_devices=8)

    # ---------------- DRAM I/O ----------------
    idx_d = nc.dram_tensor("idx", [256], mybir.dt.int32, kind="ExternalInput").ap()
    temb_d = nc.dram_tensor("temb", [V, E], BF16, kind="ExternalInput").ap()
    pos_d = nc.dram_tensor("pos", [256, E], BF16, kind="ExternalInput").ap()
    mask_d = nc.dram_tensor("masks", [12, P, P], BF16, kind="ExternalInput").ap()
    wq_d = nc.dram_tensor("wq", [L, P, KO, E], BF16, kind="ExternalInput").ap()
    wk_d = nc.dram_tensor("wk", [L, P, KO, E], BF16, kind="ExternalInput").ap()
    wv_d = nc.dram_tensor("wv", [L, P, KO, E], BF16, kind="ExternalInput").ap()
    wp_d = nc.dram_tensor("wp", [L, P, KO, E], BF16, kind="ExternalInput").ap()
    wfc_d = nc.dram_tensor("wfc", [L, 4, P, KO, E], BF16, kind="ExternalInput").ap()
    w2_d = nc.dram_tensor("w2", [L, FCK, P, E], BF16, kind="ExternalInput").ap()
    bqk_d = nc.dram_tensor("bqk", [L, 12, P], F32, kind="ExternalInput").ap()
    bfc_d = nc.dram_tensor("bfc", [L, FCK, P], F32, kind="ExternalInput").ap()
    brow_d = nc.dram_tensor("brow", [L, 1, 3 * E], F32, kind="ExternalInput").ap()
    lmw_d = nc.dram_tensor("lmw", [P, KO, VPAD], BF16, kind="ExternalInput").ap()
    lmb_d = nc.dram_tensor("lmb", [1, VPAD], F32, kind="ExternalInput").ap()
    out_d = nc.dram_tensor("logits", [1024, VS], F32, kind="ExternalOutput").ap()

    with tile.TileContext(nc) as tc:
        from contextlib import ExitStack
        gctx = ExitStack()
        # ---------------- pools ----------------
        singles = gctx.enter_context(tc.tile_pool(name="singles", bufs=1))
        pstat = gctx.enter_context(tc.tile_pool(name="pstat", bufs=4))
        pact = gctx.enter_context(tc.tile_pool(name="pact", bufs=2))
        pbias = gctx.enter_context(tc.tile_pool(name="pbias", bufs=1))
        dram = gctx.enter_context(tc.tile_pool(name="dram", bufs=1, space="DRAM"))

        # ---------------- constants / setup ----------------
        eps_sb = singles.tile([P, 1], F32, name="eps_sb")
        nc.vector.memset(eps_sb[:], EPS)
        ones_sb = singles.tile([1, P], F32, name="ones_sb")
        nc.vector.memset(ones_sb[:], 1.0)
        mask_sb = singles.tile([P, 12, P], BF16, name="mask_sb")
        nc.sync.dma_start(mask_sb[:], mask_d.rearrange("s k q -> k s q"))
        idx_sb = singles.tile([P, 2], mybir.dt.int32, name="idx_sb")
        nc.sync.dma_start(idx_sb[:], idx_d.rearrange("(c p) -> p c", p=P))
        pos_sb = singles.tile([P, 2, E], BF16, name="pos_sb")
        nc.sync.dma_start(pos_sb[:], pos_d.rearrange("(c p) m -> p c m", p=P))

        # residual stream x: [128 tok, 2 chunks, 768] fp32, persistent
        x = singles.tile([P, 2, E], F32, name="x_res")

        def layernorm(xin, xout):
            """xin fp32 [128,2,768] -> xout bf16 [128,2,768] (pure (x-m)*rstd)."""
            for c in range(2):
                st = pstat.tile([P, 2, 6], F32, tag="st")
                xv = xin[:, c, :].rearrange("p (a b) -> p a b", b=384)
                for sg in range(2):
                    nc.vector.bn_stats(st[:, sg, :], xv[:, sg, :])
                mv = pstat.tile([P, 2], F32, tag="mv")
                nc.vector.bn_aggr(mv[:], st[:])
                rstd = pstat.tile([P, 1], F32, tag="rs")
                nc.scalar.activation(rstd[:], mv[:, 1:2], AF.Sqrt, bias=eps_sb[:], scale=1.0)
                nc.vector.reciprocal(rstd[:], rstd[:])
                nmr = pstat.tile([P, 1], F32, tag="nm")
                nc.vector.tensor_tensor(nmr[:], mv[:, 0:1], rstd[:], OP.mult)
                nc.vector.tensor_scalar_mul(nmr[:], nmr[:], -1.0)
                nc.scalar.activation(xout[:, c, :], xin[:, c, :], AF.Identity,
                                     bias=nmr[:], scale=rstd[:])

        def transpose_act(xh, tag):
            """bf16 [128,2,768] token-major -> [128,6,256] feature-major."""
            xhT = pact.tile([P, KO, 256], BF16, tag=tag)
            for c in range(2):
                for f in range(KO):
                    nc.sync.dma_start_transpose(
                        xhT[:, f, ts(c, P)], xh[:, c, ts(f, P)])
            return xhT

        lctx = ExitStack()
        pw = lctx.enter_context(tc.tile_pool(name="pw", bufs=5))
        pw2 = lctx.enter_context(tc.tile_pool(name="pw2", bufs=6))
        pqkv = lctx.enter_context(tc.tile_pool(name="pqkv", bufs=2))
        pkv = lctx.enter_context(tc.tile_pool(name="pkv", bufs=1))
        patt = lctx.enter_context(tc.tile_pool(name="patt", bufs=6))
        phT = lctx.enter_context(tc.tile_pool(name="phT", bufs=1))
        ps_s = lctx.enter_context(tc.tile_pool(name="ps_s", bufs=3, space="PSUM"))
        ps_o = lctx.enter_context(tc.tile_pool(name="ps_o", bufs=2, space="PSUM"))
        ps_big = lctx.enter_context(tc.tile_pool(name="ps_big", bufs=3, space="PSUM"))
        emb_sb = singles.tile([P, 2, E], BF16, name="emb_sb")

        def embed():
            for c in range(2):
                nc.gpsimd.indirect_dma_start(
                    out=emb_sb[:, c, :], out_offset=None,
                    in_=temb_d,
                    in_offset=bass.IndirectOffsetOnAxis(ap=idx_sb[:, c:c + 1], axis=0),
                )
                nc.vector.tensor_tensor(x[:, c, :], emb_sb[:, c, :],
                                        pos_sb[:, c, :], OP.add)

        # ---------------- transformer layers ----------------
        for li in range(inner * L):
            l = li % L
            if l == 0:
                embed()
            wq_sb = pw.tile([P, KO, E], BF16, tag="w")
            nc.sync.dma_start(wq_sb[:], wq_d[l])
            wk_sb = pw.tile([P, KO, E], BF16, tag="w")
            nc.sync.dma_start(wk_sb[:], wk_d[l])
            wv_sb = pw.tile([P, KO, E], BF16, tag="w")
            nc.sync.dma_start(wv_sb[:], wv_d[l])
            wp_sb = pw.tile([P, KO, E], BF16, tag="w")
            nc.sync.dma_start(wp_sb[:], wp_d[l])
            bqk_sb = pbias.tile([P, 12], F32, tag="bqk")
            nc.sync.dma_start(bqk_sb[:], bqk_d[l].rearrange("a p -> p a"))
            bfc_sb = pbias.tile([P, FCK], F32, tag="bfc")
            nc.sync.dma_start(bfc_sb[:], bfc_d[l].rearrange("a p -> p a"))
            brow_sb = pbias.tile([1, 3 * E], F32, tag="brow")
            nc.sync.dma_start(brow_sb[:], brow_d[l])

            # LN1 + transpose
            xh = pact.tile([P, 2, E], BF16, tag="xh")
            layernorm(x, xh)
            xhT = transpose_act(xh, "xhT")

            # Q,K projections (feature-major outputs)
            qT = pqkv.tile([P, KO, 256], BF16, tag="qT")
            kT_loc = pqkv.tile([P, KO, 256], BF16, tag="kT")
            for m in range(KO):
                for dst, w_sb, boff in ((qT, wq_sb, 0), (kT_loc, wk_sb, 6)):
                    pm = ps_big.tile([P, 512], F32, tag="big")
                    for kk in range(KO):
                        nc.tensor.matmul(pm[:, :256], w_sb[:, kk, ts(m, P)],
                                         xhT[:, kk, :], start=(kk == 0), stop=(kk == 5))
                    nc.scalar.activation(dst[:, m, :], pm[:, :256], AF.Identity,
                                         bias=bqk_sb[:, boff + m:boff + m + 1], scale=1.0)

            # AllGather K
            agk_i = dram.tile([P, KO * 256], BF16, name=f"agki{li}")
            agk_o = dram.tile([4, P, KO * 256], BF16, name=f"agko{li}")
            nc.sync.dma_start(agk_i[:].rearrange("p (a b) -> p a b", a=KO), kT_loc[:])
            if _NOAG:
                for _rr in range(4):
                    nc.sync.dma_start(agk_o[_rr], agk_i[:])
            else:
                nc.gpsimd.collective_compute(
                    "AllGather", OP.bypass, replica_groups=RG,
                    ins=[agk_i[:].opt()], outs=[agk_o[:].opt()])
            kT_all = pkv.tile([P, KO, 8, P], BF16, tag="kTa")
            for rr in range(4):
                kv = agk_o[rr].rearrange("p (a b) -> p a b", a=KO)
                for cc in range(2):
                    canon = rr if cc == 0 else 7 - rr
                    nc.sync.dma_start(kT_all[:, :, canon, :], kv[:, :, ts(cc, P)])

            # V projection (token-major) + ones column
            vplus = pqkv.tile([P, 2, H, HS + 1], BF16, tag="vpl")
            nc.vector.memset(vplus[:, :, :, HS:HS + 1], 1.0)
            for tt in range(2):
                for c0, cw in ((0, 512), (512, 256)):
                    pm = ps_big.tile([P, 512], F32, tag="big")
                    nc.tensor.matmul(pm[:, :cw], ones_sb[:, :P],
                                     brow_sb[:, c0:c0 + cw], start=True, stop=False)
                    for kk in range(KO):
                        nc.tensor.matmul(pm[:, :cw], xhT[:, kk, ts(tt, P)],
                                         wv_sb[:, kk, c0:c0 + cw],
                                         start=False, stop=(kk == 5))
                    nc.vector.tensor_copy(
                        vplus[:, tt, c0 // HS:(c0 + cw) // HS, 0:HS],
                        pm[:, :cw].rearrange("p (h d) -> p h d", d=HS))

            # AllGather V
            agv_i = dram.tile([P, 2 * H * (HS + 1)], BF16, name=f"agvi{li}")
            agv_o = dram.tile([4, P, 2 * H * (HS + 1)], BF16, name=f"agvo{li}")
            nc.sync.dma_start(
                agv_i[:].rearrange("p (c h d) -> p c h d", c=2, h=H), vplus[:])
            if _NOAG:
                for _rr in range(4):
                    nc.sync.dma_start(agv_o[_rr], agv_i[:])
            else:
                nc.gpsimd.collective_compute(
                    "AllGather", OP.bypass, replica_groups=RG,
                    ins=[agv_i[:].opt()], outs=[agv_o[:].opt()])
            vplus_all = pkv.tile([P, 8, H, HS + 1], BF16, tag="vpa")
            for rr in range(4):
                vv = agv_o[rr].rearrange("p (c h d) -> p c h d", c=2, h=H)
                for cc in range(2):
                    canon = rr if cc == 0 else 7 - rr
                    nc.sync.dma_start(vplus_all[:, canon, :, :], vv[:, cc])

            # ---------------- attention ----------------
            attT = patt.tile([P, KO, 256], BF16, tag="attT", bufs=2)
            for h in range(H):
                base = HS * (h % 2)
                sub = h // 2
                psA = ps_s.tile([P, 512], F32, tag="s")
                psB1 = ps_s.tile([P, 512], F32, tag="s")
                psB2 = ps_s.tile([P, 512], F32, tag="s")
                for j in range(4):
                    nc.tensor.matmul(psA[:, ts(j, P)],
                                     kT_all[base:base + HS, sub, j, :],
                                     qT[base:base + HS, sub, 0:P],
                                     start=True, stop=True)
                for j in range(8):
                    tgt = psB1 if j < 4 else psB2
                    nc.tensor.matmul(tgt[:, ts(j % 4, P)],
                                     kT_all[base:base + HS, sub, j, :],
                                     qT[base:base + HS, sub, P:256],
                                     start=True, stop=True)
                pA = patt.tile([P, 4, P], BF16, tag="pt")
                pB1 = patt.tile([P, 4, P], BF16, tag="pt")
                pB2 = patt.tile([P, 4, P], BF16, tag="pt")
                for src, dst in ((psA, pA), (psB1, pB1), (psB2, pB2)):
                    nc.scalar.activation(dst[:], src[:].rearrange("p (a b) -> p a b", a=4),
                                         AF.Exp, scale=HS ** -0.5)
                nc.vector.tensor_tensor(pA[:], pA[:], mask_sb[:, 0:4, :], OP.mult)
                nc.vector.tensor_tensor(pB1[:], pB1[:], mask_sb[:, 4:8, :], OP.mult)
                nc.vector.tensor_tensor(pB2[:], pB2[:], mask_sb[:, 8:12, :], OP.mult)
                pso = ps_o.tile([HS + 1, 256], F32, tag="o")
                for j in range(4):
                    nc.tensor.matmul(pso[:, 0:P], vplus_all[:, j, h, :],
                                     pA[:, j, :], start=(j == 0), stop=(j == 3))
                for j in range(8):
                    pt = pB1 if j < 4 else pB2
                    nc.tensor.matmul(pso[:, P:256], vplus_all[:, j, h, :],
                                     pt[:, j % 4, :], start=(j == 0), stop=(j == 7))
                rc = patt.tile([1, 256], F32, tag="rc", bufs=4)
                nc.vector.reciprocal(rc[:], pso[HS:HS + 1, :])
                rcd = dram.tile([1, 256], F32, tag="rcd", bufs=2)
                nc.sync.dma_start(rcd[:], rc[:])
                rcb = patt.tile([HS, 256], F32, tag="rcb", bufs=2)
                rcd_ap = rcd[:]
                bcast = bass.AP(tensor=rcd_ap.tensor, offset=rcd_ap.offset,
                                ap=[[0, HS]] + [list(p) for p in rcd_ap.ap][1:])
                nc.sync.dma_start(rcb[:], bcast)
                if base == 0:
                    nc.vector.tensor_tensor(attT[0:HS, sub, :], pso[0:HS, :],
                                            rcb[:], OP.mult)
                else:
                    ot = patt.tile([HS, 256], BF16, tag="ot", bufs=2)
                    nc.vector.tensor_tensor(ot[:], pso[0:HS, :], rcb[:], OP.mult)
                    nc.sync.dma_start(attT[HS:P, sub, :], ot[:])

            # output projection + residual
            for tt in range(2):
                for c0, cw in ((0, 512), (512, 256)):
                    pm = ps_big.tile([P, 512], F32, tag="big")
                    nc.tensor.matmul(pm[:, :cw], ones_sb[:, :P],
                                     brow_sb[:, E + c0:E + c0 + cw], start=True, stop=False)
                    for kk in range(KO):
                        nc.tensor.matmul(pm[:, :cw], attT[:, kk, ts(tt, P)],
                                         wp_sb[:, kk, c0:c0 + cw],
                                         start=False, stop=(kk == 5))
                    nc.vector.tensor_tensor(x[:, tt, c0:c0 + cw], x[:, tt, c0:c0 + cw],
                                            pm[:, :cw], OP.add)

            # ---------------- FFN ----------------
            xh2 = pact.tile([P, 2, E], BF16, tag="xh")
            layernorm(x, xh2)
            xh2T = transpose_act(xh2, "xhT")
            hT = phT.tile([P, FCK, 256], BF16, tag="hT")
            for ci in range(4):
                wfc_sb = pw.tile([P, KO, E], BF16, tag="w")
                nc.sync.dma_start(wfc_sb[:], wfc_d[l, ci])
                for mm in range(KO):
                    ch = ci * KO + mm
                    pm = ps_big.tile([P, 512], F32, tag="big")
                    for kk in range(KO):
                        nc.tensor.matmul(pm[:, :256], wfc_sb[:, kk, ts(mm, P)],
                                         xh2T[:, kk, :], start=(kk == 0), stop=(kk == 5))
                    nc.scalar.activation(hT[:, ch, :], pm[:, :256], AF.Relu,
                                         bias=bfc_sb[:, ch:ch + 1], scale=1.0)
            for c0, cw in ((0, 512), (512, 256)):
                pms = []
                for tt in range(2):
                    pm = ps_big.tile([P, 512], F32, tag="big")
                    nc.tensor.matmul(pm[:, :cw], ones_sb[:, :P],
                                     brow_sb[:, 2 * E + c0:2 * E + c0 + cw],
                                     start=True, stop=False)
                    pms.append(pm)
                for kk in range(FCK):
                    w2_sb = pw2.tile([P, 512], BF16, tag="w2")
                    nc.sync.dma_start(w2_sb[:, :cw], w2_d[l, kk][:, c0:c0 + cw])
                    for tt in range(2):
                        nc.tensor.matmul(pms[tt][:, :cw], hT[:, kk, ts(tt, P)],
                                         w2_sb[:, :cw], start=False, stop=(kk == FCK - 1))
                for tt in range(2):
                    nc.vector.tensor_tensor(x[:, tt, c0:c0 + cw], x[:, tt, c0:c0 + cw],
                                            pms[tt][:, :cw], OP.add)

        # ---------------- final LN + AllGather + lm_head ----------------
        xfTs = []
        for frep in range(inner):
            xhf = pact.tile([P, 2, E], BF16, tag="xh")
            layernorm(x, xhf)
            xhfT = transpose_act(xhf, "xhT")
            agf_i = dram.tile([P, KO * 256], BF16, name=f"agfi{frep}")
            agf_o = dram.tile([4, P, KO * 256], BF16, name=f"agfo{frep}")
            nc.sync.dma_start(agf_i[:].rearrange("p (a b) -> p a b", a=KO), xhfT[:])
            if _NOAG:
                for _rr in range(4):
                    nc.sync.dma_start(agf_o[_rr], agf_i[:])
            else:
                nc.gpsimd.collective_compute(
                    "AllGather", OP.bypass, replica_groups=RG,
                    ins=[agf_i[:].opt()], outs=[agf_o[:].opt()])
            xfT = pkv.tile([P, KO, 8, P], BF16, tag="kTa")
            for rr in range(4):
                kv = agf_o[rr].rearrange("p (a b) -> p a b", a=KO)
                for cc in range(2):
                    canon = rr if cc == 0 else 7 - rr
                    nc.sync.dma_start(xfT[:, :, canon, :], kv[:, :, ts(cc, P)])
            xfTs.append(xfT)

        lctx.close()

        with tc.tile_pool(name="plm", bufs=3) as plm, \
             tc.tile_pool(name="plog", bufs=6) as plog, \
             tc.tile_pool(name="ps_lm", bufs=6, space="PSUM") as ps_lm:
          for frep in range(inner):
            xfT = xfTs[frep]
            for chk in range(NLM):
                lw = plm.tile([P, KO, 512], BF16, tag="lw")
                N = 512 if chk < NLM - 1 else VS - 512 * (NLM - 1)
                nc.sync.dma_start(lw[:], lmw_d[:, :, ts(chk, 512)])
                lmb_sb = plm.tile([1, 512], F32, tag="lmb")
                nc.sync.dma_start(lmb_sb[:, :N], lmb_d[:, 512 * chk:512 * chk + N])
                for tt in range(8):
                    pm = ps_lm.tile([P, 512], F32, tag="lm")
                    nc.tensor.matmul(pm[:, :N], ones_sb[:, :P],
                                     lmb_sb[:, :N], start=True, stop=False)
                    for kk in range(KO):
                        nc.tensor.matmul(pm[:, :N], xfT[:, kk, tt, :],
                                         lw[:, kk, :N], start=False, stop=(kk == 5))
                    lg = plog.tile([P, 512], F32, tag="lg")
                    if tt % 2 == 0:
                        nc.scalar.copy(lg[:, :N], pm[:, :N])
                    else:
                        nc.vector.tensor_copy(lg[:, :N], pm[:, :N])
                    nc.sync.dma_start(out_d[ts(tt, P), 512 * chk:512 * chk + N],
                                      lg[:, :N])
        gctx.close()

    nc.compile()
    return nc


def _prep(inputs):
    bf = ml_dtypes.bfloat16
    f = np.float32
    g = lambda k: np.asarray(inputs[k], f)
    idx = np.asarray(inputs["idx"]).astype(np.int32)
    wq, wk, wv, wproj = g("wq"), g("wk"), g("wv"), g("wproj")
    g1, b1, g2, b2 = g("ln1_g"), g("ln1_b"), g("ln2_g"), g("ln2_b")
    wfc, wpr2 = g("wfc"), g("wpr2")
    bfc, bproj, bpr2 = g("bfc"), g("bproj"), g("bpr2")
    gf, bff = g("lnf_g"), g("lnf_b")
    lm_w, lm_b = g("lm_w"), g("lm_b")

    wq_e = g1[:, :, None] * wq
    wk_e = g1[:, :, None] * wk
    wv_e = g1[:, :, None] * wv
    wfc_e = g2[:, :, None] * wfc
    bq_e = np.einsum("le,leo->lo", b1, wq)
    bk_e = np.einsum("le,leo->lo", b1, wk)
    bv_e = np.einsum("le,leo->lo", b1, wv)
    bfc_e = bfc + np.einsum("le,leo->lo", b2, wfc)
    lmw_e = gf[:, None] * lm_w
    lmb_e = lm_b + bff @ lm_w

    def pack(w):  # [L,768,N] -> [L,128,6,N]
        Lx, Ex, Nx = w.shape
        return np.ascontiguousarray(
            w.reshape(Lx, KO, P, Nx).transpose(0, 2, 1, 3)).astype(bf)

    com = {
        "temb": np.asarray(inputs["tok_emb"], f).astype(bf),
        "wq": pack(wq_e), "wk": pack(wk_e), "wv": pack(wv_e), "wp": pack(wproj),
        "wfc": np.ascontiguousarray(
            pack(wfc_e).reshape(L, P, KO, 4, E).transpose(0, 3, 1, 2, 4)),
        "w2": wpr2.reshape(L, FCK, P, E).astype(bf),
        "bqk": np.concatenate(
            [bq_e.reshape(L, KO, P), bk_e.reshape(L, KO, P)], axis=1).astype(f),
        "bfc": bfc_e.reshape(L, FCK, P).astype(f),
        "brow": np.concatenate([bv_e, bproj, bpr2], axis=1)[:, None, :].astype(f),
    }
    pos = np.asarray(inputs["pos_emb"], f).astype(bf)

    in_maps = []
    ar = np.arange(P)
    for core in range(8):
        gb, r = divmod(core, 4)
        c1, c2 = r, 7 - r
        m = dict(com)
        sl = lmw_e[:, r * VS:(r + 1) * VS]
        lmw_pad = np.zeros((E, VPAD), f)
        lmw_pad[:, :VS] = sl
        m["lmw"] = np.ascontiguousarray(
            lmw_pad.reshape(KO, P, VPAD).transpose(1, 0, 2)).astype(bf)
        lmb_pad = np.zeros((1, VPAD), f)
        lmb_pad[0, :VS] = lmb_e[r * VS:(r + 1) * VS]
        m["lmb"] = lmb_pad
        m["idx"] = np.concatenate(
            [idx[gb, c1 * P:(c1 + 1) * P], idx[gb, c2 * P:(c2 + 1) * P]])
        m["pos"] = np.concatenate(
            [pos[c1 * P:(c1 + 1) * P], pos[c2 * P:(c2 + 1) * P]])
        masks = np.zeros((12, P, P), f)
        for j in range(4):
            masks[j] = (j * P + ar[:, None]) <= (c1 * P + ar[None, :])
        for j in range(8):
            masks[4 + j] = (j * P + ar[:, None]) <= (c2 * P + ar[None, :])
        m["masks"] = masks.astype(bf)
        in_maps.append(m)
    return in_maps


LAST_RESULTS = None
LAST_TIME_NS = None


def _timed_run(nc, in_maps, reps=3):
    """Replicates bass2jax.run_bass_via_pjrt's multi-core path, but keeps
    inputs device-resident so repeated executions time the NEFF itself."""
    import time as _time
    import jax
    from jax.experimental.shard_map import shard_map
    from jax.sharding import Mesh, PartitionSpec, NamedSharding
    from concourse import bass2jax as b2j
    import concourse.mybir as _mb

    b2j.install_neuronx_cc_hook()
    n_cores = len(in_maps)
    partition_name = nc.partition_id_tensor.name if nc.partition_id_tensor else None
    in_names, out_names, out_avals, zero_outs = [], [], [], []
    for alloc in nc.m.functions[0].allocations:
        if not isinstance(alloc, _mb.MemoryLocationSet):
            continue
        name = alloc.memorylocations[0].name
        if alloc.kind == "ExternalInput":
            if name != partition_name:
                in_names.append(name)
        elif alloc.kind == "ExternalOutput":
            out_names.append(name)
            shape = tuple(alloc.tensor_shape)
            dtype = _mb.dt.np(alloc.dtype)
            out_avals.append(jax.core.ShapedArray(shape, dtype))
            zero_outs.append(np.zeros(shape, dtype))
    n_params = len(in_names)
    n_outs = len(out_avals)
    in_names.extend(out_names)
    if partition_name is not None:
        in_names.append(partition_name)
    donate = tuple(range(n_params, n_params + n_outs))

    def _body(*args):
        operands = list(args)
        if partition_name is not None:
            operands.append(b2j.partition_id_tensor())
        return tuple(b2j._bass_exec_p.bind(
            *operands, out_avals=tuple(out_avals), in_names=tuple(in_names),
            out_names=tuple(out_names), lowering_input_output_aliases=(),
            sim_require_finite=True, sim_require_nnan=True, nc=nc))

    devices = jax.devices()[:n_cores]
    mesh = Mesh(np.asarray(devices), ("core",))
    spec = PartitionSpec("core")
    sharded = jax.jit(
        shard_map(_body, mesh=mesh, in_specs=(spec,) * (n_params + n_outs),
                  out_specs=(spec,) * n_outs, check_rep=False),
        donate_argnums=donate, keep_unused=True)
    sh = NamedSharding(mesh, spec)
    concat_in = [
        jax.device_put(
            np.concatenate([np.asarray(in_maps[c][nm]) for c in range(n_cores)], axis=0),
            sh)
        for nm in in_names[:n_params]]
    jax.block_until_ready(concat_in)
    times = []
    out_arrs = None
    for rep in range(reps):
        zeros_dev = [
            jax.device_put(np.zeros((n_cores * z.shape[0], *z.shape[1:]), z.dtype), sh)
            for z in zero_outs]
        jax.block_until_ready(zeros_dev)
        t0 = _time.perf_counter()
        out_arrs = sharded(*concat_in, *zeros_dev)
        jax.block_until_ready(out_arrs)
        times.append(_time.perf_counter() - t0)
    results = [
        {nm: np.asarray(out_arrs[i]).reshape(n_cores, *out_avals[i].shape)[c]
         for i, nm in enumerate(out_names)}
        for c in range(n_cores)]
    return results, times


def kernel(**inputs):
    global LAST_RESULTS, LAST_TIME_NS
    import os
    inner = int(os.environ.get("KBENCH_INNER", "1"))
    if ("nc", inner) not in _cache:
        _cache[("nc", inner)] = _build(inner)
    nc = _cache[("nc", inner)]
    in_maps = _prep(inputs)
    reps = int(os.environ.get("KBENCH_TIME_REPS", "0"))
    if reps > 0:
        results, times = _timed_run(nc, in_maps, reps=reps)
        LAST_TIME_NS = int(min(times) * 1e9)
        LAST_RESULTS = None
    else:
        res = run_bass_kernel_spmd(nc, in_maps, core_ids=list(range(8)))
        LAST_RESULTS = res
        results = res.results
    out = np.zeros((B, T, V), np.float32)
    for core in range(8):
        gb, r = divmod(core, 4)
        out[gb, :, r * VS:(r + 1) * VS] = results[core]["logits"]
    return out
